# revision 24
# baseline (speedup 1.0000x reference)
"""Causal multi-head self-attention on 8 TRN2 NeuronCores, tensor-parallel
over heads.

Reference: x:(4,2048,1024) f32, Wq/Wk/Wv/Wo:(1024,1024) f32, 16 heads, d_k=64.

Sharding: each core owns 2 heads (128 of the 1024 q/k/v/attn-out dims).
Per core: QKV projections for its head slice, causal attention for its
8 (batch, head) units, and a partial output projection against its 128
columns of Wo. The 8 partial outputs are summed on the host (the
tensor-parallel unshard), so no on-device collective is needed.

Device layouts are feature-major ("transposed"): the host passes x.T and
pre-transposed weight slices so every matmul contraction dim lands on the
SBUF partition axis. Scores are computed as S.T = K @ Q.T per 128-key x
512-query tile (both heads row-tiled into disjoint PE row groups); exp is
fused into the PSUM->SBUF evacuation on the scalar engine; causal masking
multiplies the diagonal tiles by a 0/1 mask after exp; softmax
denominators come from a ones-column appended to V (the attn @ V matmul
also produces the row sums); the per-query reciprocal is exp(-ln(sum)) on
the scalar engine (one activation-table set covers Exp+Ln), broadcast
across partitions via a K=1 matmul against ones.
"""
import numpy as np

# ---------------------------------------------------------------------------
# Workaround for this walrus build's sync-wait capacity limit: it rejects any
# regular instruction carrying more than 1 sem wait (EventSemaphore carries 2),
# while Tile's add_semaphores stage freely attaches several. After the build we
# rewrite every basic block, moving excess waits onto InstEventSemaphore
# instructions inserted immediately before the owning instruction on the same
# engine queue (identical semantics: the engine blocks until all waits pass).
import concourse.mybir as mybir

_EVN = [0]


def _split_excess_waits(nc):
    for f in nc.m.functions:
        for bb in f.blocks:
            insts = bb.instructions
            new_list = []
            changed = False
            for inst in insts:
                si = inst.sync_info
                waits = list(si.on_wait) if si and si.on_wait else []
                cap = 2 if isinstance(inst, mybir.InstEventSemaphore) else 1
                if len(waits) > cap:
                    changed = True
                    extra, keep = waits[cap:], waits[:cap]
                    for kk in range(0, len(extra), 2):
                        _EVN[0] += 1
                        ev = mybir.InstEventSemaphore(
                            name=f"evsplit-{_EVN[0]}",
                            opcode="EventSemaphore",
                            engine=inst.engine,
                            sync_info=mybir.SyncInfo(
                                on_wait=extra[kk : kk + 2], on_update=[]
                            ),
                        )
                        nc.register_instruction(ev, overwrite=True)
                        new_list.append(ev)
                    inst.sync_info = mybir.SyncInfo(
                        on_wait=keep, on_update=list(si.on_update or [])
                    )
                new_list.append(inst)
            if changed:
                insts[:] = new_list
    return nc


import concourse.bass as bass
import concourse.tile as tile
from concourse.bass_utils import run_bass_kernel_spmd
from concourse.masks import make_identity

F32 = mybir.dt.float32
F32R = mybir.dt.float32r
BF16 = mybir.dt.bfloat16

B = 4          # batches
S = 2048       # sequence length
D = 1024       # d_model
DK = 64        # head dim
NCORES = 8
HPC = 2        # heads per core
HD = HPC * DK  # 128: per-core q/k/v/attn-out dims
TB = 512       # token block (matmul moving free dim)
NTB = S // TB  # 4 token blocks per batch
NKC = S // 128  # 16 key chunks per batch
SCALE = 1.0 / np.sqrt(DK)

_BUILT = None  # built Bass graph cache — building/scheduling is expensive


def _build():
    nc = bass.Bass()
    xT = nc.declare_dram_parameter("xT", [128, 8, B * S], BF16, isOutput=False)
    wqT = nc.declare_dram_parameter("wqT", [128, 8, HD], BF16, isOutput=False)
    wkT = nc.declare_dram_parameter("wkT", [128, 8, HD], BF16, isOutput=False)
    wvT = nc.declare_dram_parameter("wvT", [128, 8, HD], BF16, isOutput=False)
    woT = nc.declare_dram_parameter("woT", [HD, D], BF16, isOutput=False)
    masks = nc.declare_dram_parameter("masks", [128, 4, TB], BF16, isOutput=False)
    out = nc.declare_dram_parameter("out", [D, B * S], BF16, isOutput=True)

    with tile.TileContext(nc) as tc:
        with (
            tc.tile_pool(name="const", bufs=1) as cpool,
            tc.tile_pool(name="xin", bufs=3) as xpool,
            tc.tile_pool(name="qk", bufs=2) as qkpool,
            tc.tile_pool(name="vsb", bufs=2) as vpool,
            tc.tile_pool(name="vt", bufs=2) as vtpool,
            tc.tile_pool(name="pt", bufs=8) as ptpool,
            tc.tile_pool(name="ot", bufs=2) as otpool,
            tc.tile_pool(name="oev", bufs=3) as opool,
            tc.tile_pool(name="rc", bufs=4) as rpool,
            tc.tile_pool(name="dscr", bufs=4, space="DRAM") as dpool,
            tc.tile_pool(name="pmisc", bufs=2, space="PSUM") as pmisc,
            tc.tile_pool(name="pscore", bufs=2, space="PSUM") as pscore,
            tc.tile_pool(name="pav", bufs=2, space="PSUM") as pav,
        ):
            # --- constants / weights (resident). wo/mask/ones loads are
            # emitted after batch-0's projections to keep the startup window
            # free for the first x block + QKV weights.
            wq_sb = cpool.tile([128, 8, HD], BF16, tag="wq")
            nc.sync.dma_start(wq_sb[:], wqT.ap())
            wk_sb = cpool.tile([128, 8, HD], BF16, tag="wk")
            nc.sync.dma_start(wk_sb[:], wkT.ap())
            wv_sb = cpool.tile([128, 8, HD], BF16, tag="wv")
            nc.sync.dma_start(wv_sb[:], wvT.ap())
            wo_sb = cpool.tile([HD, D], BF16, tag="wo")
            mask_sb = cpool.tile([128, 4, TB], BF16, tag="mask")
            ident = cpool.tile([128, 128], BF16, tag="ident")
            make_identity(nc, ident[:])

            def emit_qkv(b):
                """QKV projections for batch b; returns (qt, kt, v) tiles."""
                base = b * S
                qt_sb = qkpool.tile([128, S], BF16, tag="QT", name=f"qt{b}")
                kt_sb = qkpool.tile([128, S], BF16, tag="KT", name=f"kt{b}")
                v_sb = vpool.tile(
                    [128, NKC, HPC, DK + 1], BF16, tag="VSB", name=f"v{b}"
                )
                for tb in range(NTB):
                    tok = base + tb * TB
                    x_t = xpool.tile([128, 8, TB], BF16, tag="XT", name="xt")
                    nc.sync.dma_start(x_t[:], xT.ap()[:, :, tok : tok + TB])
                    for w_sb, dst in ((wq_sb, qt_sb), (wk_sb, kt_sb)):
                        ps = pmisc.tile([128, TB], F32, tag="pp", name="psqk")
                        for c in range(8):
                            nc.tensor.matmul(
                                ps[:],
                                w_sb[:, c, :],
                                x_t[:, c, :],
                                start=(c == 0),
                                stop=(c == 7),
                            )
                        nc.vector.tensor_copy(
                            dst[:, tb * TB : (tb + 1) * TB], ps[:]
                        )
                    psv = pmisc.tile([128, TB], F32, tag="pp", name="psv")
                    for c in range(8):
                        nc.tensor.matmul(
                            psv[:],
                            wv_sb[:, c, :],
                            x_t[:, c, :],
                            start=(c == 0),
                            stop=(c == 7),
                        )
                    vt_t = vtpool.tile([128, TB], BF16, tag="VT", name="vt")
                    nc.vector.tensor_copy(vt_t[:], psv[:])
                    for j in range(TB // 128):
                        kc = tb * (TB // 128) + j
                        pst = pmisc.tile([128, 128], BF16, tag="pp", name="pst")
                        nc.tensor.transpose(
                            pst[:], vt_t[:, j * 128 : (j + 1) * 128], ident[:]
                        )
                        nc.vector.tensor_copy(
                            v_sb[:, kc, :, 0:DK],
                            pst[:].rearrange("p (h d) -> p h d", h=HPC),
                        )
                for h in range(HPC):
                    nc.vector.memset(v_sb[:, :, h, DK], 1.0)
                return qt_sb, kt_sb, v_sb

            def emit_outproj_block(b, ot_sb, tb):
                base = b * S
                tok = base + tb * TB
                for oc in range(D // 128):
                    ps_o = pmisc.tile([128, TB], F32, tag="pp", name="pso")
                    nc.tensor.matmul(
                        ps_o[:],
                        wo_sb[:, oc * 128 : (oc + 1) * 128],
                        ot_sb[:, tb * TB : (tb + 1) * TB],
                        start=True,
                        stop=True,
                    )
                    o_t = opool.tile([128, TB], BF16, tag="OE")
                    nc.vector.tensor_copy(o_t[:], ps_o[:])
                    nc.sync.dma_start(
                        out.ap()[oc * 128 : (oc + 1) * 128, tok : tok + TB],
                        o_t[:],
                    )

            def emit_attention(b, qt_sb, kt_sb, v_sb, reverse=False):
                """Causal attention for batch b; returns the (normalized)
                attn-output tile OT [128, S]. reverse=True processes the
                largest query block first so the kernel tail is gated on the
                smallest one."""
                ot_sb = otpool.tile([128, S], BF16, tag="OT", name=f"ot{b}")
                qb_order = range(NTB - 1, -1, -1) if reverse else range(NTB)
                for qb in qb_order:
                    ps_av = [
                        pav.tile([128, TB], F32, tag="pav", name=f"pav{h}")
                        for h in range(HPC)
                    ]
                    nkc = (qb + 1) * (TB // 128)

                    def emit_scores(kc):
                        # Diagonal tiles only need queries >= their first key:
                        # shorten the moving dim to the causally-valid column
                        # range (q0..TB); earlier columns finished accumulating
                        # in previous key chunks.
                        j = kc - 4 * qb
                        q0 = max(j, 0) * 128
                        qs = slice(qb * TB + q0, (qb + 1) * TB)
                        ps_s = pscore.tile([128, HPC, TB], F32, tag="ps")
                        for h in range(HPC):
                            nc.tensor.matmul(
                                ps_s[:, h, q0:],
                                kt_sb[h * DK : (h + 1) * DK, kc * 128 : (kc + 1) * 128],
                                qt_sb[h * DK : (h + 1) * DK, qs],
                                start=True,
                                stop=True,
                                tile_position=(h * DK, 0),
                            )
                        pt = ptpool.tile([128, HPC, TB], BF16, tag="PT")
                        nc.scalar.activation(
                            pt[:, :, q0:], ps_s[:, :, q0:],
                            mybir.ActivationFunctionType.Exp, scale=SCALE,
                        )
                        if j >= 0:  # diagonal tile: zero the non-causal region
                            nc.vector.tensor_tensor(
                                pt[:, :, q0:],
                                pt[:, :, q0:],
                                mask_sb[:, j : j + 1, q0:].to_broadcast(
                                    [128, HPC, TB - q0]
                                ),
                                mybir.AluOpType.mult,
                            )
                        return pt, q0

                    def emit_av(kc, pt, q0):
                        for h in range(HPC):
                            nc.tensor.matmul(
                                ps_av[h][0 : DK + 1, q0:],
                                v_sb[:, kc, h, :],
                                pt[:, h, q0:],
                                start=(kc == 0),
                                stop=(kc == nkc - 1),
                            )

                    # 1-deep software pipeline: the next chunk's score matmuls
                    # sit before this chunk's AV matmuls in the PE's static
                    # order, hiding the exp latency between them.
                    prev = None
                    for kc in range(nkc):
                        cur = (kc, *emit_scores(kc))
                        if prev is not None:
                            emit_av(*prev)
                        prev = cur
                    emit_av(*prev)
                    for h in range(HPC):
                        # Evacuate the AV accumulator immediately so the PSUM
                        # bank frees for the next query block; normalize from
                        # SBUF afterwards. 1/rowsum is exp(-ln(sum)) on ScalarE
                        # (same act-table set as the score exp).
                        ou_sb = rpool.tile(
                            [DK + 1, TB], F32, tag="ou", name=f"ou{h}"
                        )
                        nc.vector.tensor_copy(ou_sb[:], ps_av[h][0 : DK + 1, :])
                        ln_s = rpool.tile([1, TB], F32, tag="lns", name=f"lns{h}")
                        nc.scalar.activation(
                            ln_s[:], ou_sb[DK : DK + 1, :],
                            mybir.ActivationFunctionType.Ln,
                        )
                        recr = rpool.tile([1, TB], F32, tag="recr", name=f"recr{h}")
                        nc.scalar.activation(
                            recr[:], ln_s[:],
                            mybir.ActivationFunctionType.Exp, scale=-1.0,
                        )
                        # broadcast 1/sum across partitions with a DRAM
                        # round-trip (DRAM APs may repeat a row; engines
                        # cannot partition-broadcast from SBUF)
                        scr = dpool.tile([1, TB], F32, tag="scr", name=f"scr{h}")
                        nc.sync.dma_start(scr[:], recr[:])
                        rb_sb = rpool.tile([DK, TB], F32, tag="rb", name=f"rb{h}")
                        nc.sync.dma_start(
                            rb_sb[:], scr[:].to_broadcast([DK, TB])
                        )
                        nc.vector.tensor_tensor(
                            ot_sb[h * DK : (h + 1) * DK, qb * TB : (qb + 1) * TB],
                            ou_sb[0:DK, :],
                            rb_sb[:],
                            mybir.AluOpType.mult,
                        )
                return ot_sb

            def emit_outproj(b, ot_sb, reverse=False):
                tb_order = range(NTB - 1, -1, -1) if reverse else range(NTB)
                for tb in tb_order:
                    emit_outproj_block(b, ot_sb, tb)

            # Software-pipelined emission: the PE's static instruction order
            # places batch b+1's projections BEFORE batch b's output
            # projection, so the PE has independent work while the last
            # query block's softmax-normalization DMA round-trip completes.
            tiles = emit_qkv(0)
            nc.sync.dma_start(mask_sb[:], masks.ap())
            nc.sync.dma_start(wo_sb[:], woT.ap())
            for b in range(B):
                last = b == B - 1
                ot = emit_attention(b, *tiles, reverse=last)
                if not last:
                    tiles = emit_qkv(b + 1)
                emit_outproj(b, ot, reverse=last)

    _split_excess_waits(nc)
    return nc


def _host_inputs(x, Wq, Wk, Wv, Wo):
    """Shard + lay out the full inputs for the 8 cores."""
    import ml_dtypes
    bf = ml_dtypes.bfloat16
    xt = np.ascontiguousarray(
        x.reshape(B * S, D).T.reshape(8, 128, B * S).transpose(1, 0, 2)
    ).astype(bf)  # [128, 8, B*S], feature-major
    col = np.arange(TB)[None, :]
    row = np.arange(128)[:, None]
    masks = np.stack(
        [(col >= row + j * 128).astype(np.float32) for j in range(4)], axis=1
    ).astype(bf)  # [128, 4, TB] 0/1

    def wslice(W, c):  # [128, 8, HD] chunk-major W[c*HD:(c+1)*HD, :].T
        wt = W[c * HD : (c + 1) * HD, :].T  # (D, HD)
        return np.ascontiguousarray(
            wt.reshape(8, 128, HD).transpose(1, 0, 2)
        ).astype(bf)

    in_maps = []
    for c in range(NCORES):
        in_maps.append(
            {
                "xT": xt,
                "wqT": wslice(Wq, c),
                "wkT": wslice(Wk, c),
                "wvT": wslice(Wv, c),
                "woT": np.ascontiguousarray(
                    Wo[:, c * HD : (c + 1) * HD].T
                ).astype(bf),
                "masks": masks,
            }
        )
    return in_maps


def run(x, Wq, Wk, Wv, Wo, trace=False):
    """Run the SPMD kernel; returns (output, BassKernelResults)."""
    global _BUILT
    if _BUILT is None:
        _BUILT = _build()
    nc = _BUILT
    in_maps = _host_inputs(
        np.asarray(x, dtype=np.float32),
        np.asarray(Wq, dtype=np.float32),
        np.asarray(Wk, dtype=np.float32),
        np.asarray(Wv, dtype=np.float32),
        np.asarray(Wo, dtype=np.float32),
    )
    res = run_bass_kernel_spmd(
        nc, in_maps, core_ids=list(range(NCORES)), trace=trace
    )
    acc = np.zeros((D, B * S), dtype=np.float32)
    for c in range(NCORES):
        acc += res.results[c]["out"].astype(np.float32)
    out = np.ascontiguousarray(acc.T).reshape(B, S, D)
    return out, res


def kernel(x, Wq, Wk, Wv, Wo):
    out, _ = run(x, Wq, Wk, Wv, Wo, trace=False)
    return out
